# revision 5
# baseline (speedup 1.0000x reference)
"""DBRX MoE experts kernel for 8 Trainium2 NeuronCores — v3.

Expert-parallel (E=8 == n_cores), fp16 matmul inputs, fp32 PSUM accumulation.
Tokens are split into two resident halves; per half, phase A loops m-chunks
with w1/v1 loaded once and token blocks INNER (each stationary 128x128
weight chunk feeds all token blocks back-to-back), producing
P = silu(x@w1) * (x@v1) fp16 resident in SBUF; phase B accumulates
y^T = w2^T P over all 32 h-chunks with f-chunk groups of 2 and token blocks
inner (again reusing each stationary w2 chunk across blocks).

v3 (from TimelineSim gap analysis): the m=0 weight DMAs are issued right
after the first x tile instead of behind the whole 4MB x fill, cutting the
kernel-start PE idle from ~16us to ~5us. Measured (same-session interleaved
A/B, hw-loop slope): 1319.9us -> 1311.3us per run; the PE-streaming roofline
for C=1992 at 2.4GHz is 1274.9us and the cost-model floor incl. edges is
~1295us, so the schedule is within ~1.3% of the modeled optimum. fp8
(DoubleRow) was evaluated and rejected: measured end-to-end rel_err 2.7e-2..
6.6e-2 for every fp8 placement vs the 2e-2 gate (fp16 all: 5.5e-4).

Host dispatches tokens to experts (gather), applies combine weights, and
scatter-adds expert outputs.
"""
import numpy as np

F = 2048          # token feature dim
H = 4096          # expert hidden dim
E = 8             # experts == cores
CB = 512          # token block width (PSUM bank, fp32)
KC = F // 128     # 16 k-chunks (contraction for phase A; f-chunks for y)
MH = H // 128     # 32 m-chunks (hidden; contraction for phase B)
FGB = 2           # f-chunks per phase-B group

_cache = {}

# NB: tile-pool `bufs` is PER TAG. xt uses 16 tags (one per k-chunk), so
# bufs=2 double-buffers each across halves: 16*2*2.25KB = 72KB/partition.
BUFS = {"xt": 2, "w1": 4, "v1": 4, "w2": 10, "p": MH, "tmp": 8, "y": 6}


def _round_tf32(x: np.ndarray) -> np.ndarray:
    return np.ascontiguousarray(x).astype(np.float16)


def _blocks(ch):
    ws = []
    c0 = 0
    while c0 < ch:
        w = min(CB, ch - c0)
        ws.append((c0, w))
        c0 += w
    return ws


def _halves(C):
    # chunks of at most 2 blocks (1024 tokens): keeps phase-A PSUM use at
    # 2*nb<=4+4 banks and the resident x/P tiles within SBUF for any C
    out = []
    c0 = 0
    while c0 < C:
        ch = min(2 * CB, C - c0)
        out.append((c0, ch))
        c0 += ch
    return out


def _build(C, loop_r=None):
    import contextlib
    import concourse.mybir as mybir
    import concourse.tile as tile
    from concourse import bacc

    F32 = mybir.dt.float32
    F16 = mybir.dt.float16
    Silu = mybir.ActivationFunctionType.Silu

    nc = bacc.Bacc("TRN2", target_bir_lowering=False, debug=False)
    xt = nc.dram_tensor("xt", [KC, 128, C], F16, kind="ExternalInput").ap()
    w1t = nc.dram_tensor("w1t", [MH, 128, KC, 128], F16, kind="ExternalInput").ap()
    v1t = nc.dram_tensor("v1t", [MH, 128, KC, 128], F16, kind="ExternalInput").ap()
    w2t = nc.dram_tensor("w2t", [MH, 128, F], F16, kind="ExternalInput").ap()
    yt = nc.dram_tensor("yt", [KC, 128, C], F32, kind="ExternalOutput").ap()

    with tile.TileContext(nc) as tc:
        with tc.tile_pool(name="xtp", bufs=BUFS["xt"]) as xt_pool, \
             tc.tile_pool(name="w1p", bufs=BUFS["w1"]) as w1_pool, \
             tc.tile_pool(name="v1p", bufs=BUFS["v1"]) as v1_pool, \
             tc.tile_pool(name="w2p", bufs=BUFS["w2"]) as w2_pool, \
             tc.tile_pool(name="pp", bufs=BUFS["p"]) as p_pool, \
             tc.tile_pool(name="tmpp", bufs=BUFS["tmp"]) as tmp_pool, \
             tc.tile_pool(name="yp", bufs=BUFS["y"]) as y_pool, \
             tc.tile_pool(name="ps", bufs=8, space="PSUM") as psum, \
             (tc.For_i(0, loop_r, 1) if loop_r else contextlib.nullcontext()):
            for hi, (h0, ch) in enumerate(_halves(C)):
                blocks = _blocks(ch)
                # resident x^T tiles for this half. Issue xt[0] first, then
                # m=0's weights, then the rest of xt: the PE can then start
                # the k=0 matmuls ~2.5us in instead of waiting out the full
                # 4MB xt fill before w1m even starts (saves ~12us startup).
                xts = []
                w01 = None
                for k in range(KC):
                    t = xt_pool.tile([128, ch], F16, tag=f"xt{k}",
                                     name=f"xt{hi}_{k}")
                    nc.sync.dma_start(t[:], xt[k][:, h0:h0 + ch])
                    xts.append(t)
                    if k == 0:
                        w1m0 = w1_pool.tile([128, KC * 128], F16, tag="w1")
                        nc.sync.dma_start(
                            w1m0[:], w1t[0].rearrange("p k j -> p (k j)"))
                        v1m0 = v1_pool.tile([128, KC * 128], F16, tag="v1")
                        nc.sync.dma_start(
                            v1m0[:], v1t[0].rearrange("p k j -> p (k j)"))
                        w01 = (w1m0, v1m0)

                # ---- Phase A: P[m] = silu(w1^T x^T) * (v1^T x^T)
                ptiles = []
                for m in range(MH):
                    if m == 0:
                        w1m, v1m = w01
                    else:
                        w1m = w1_pool.tile([128, KC * 128], F16, tag="w1")
                        nc.sync.dma_start(
                            w1m[:], w1t[m].rearrange("p k j -> p (k j)"))
                        v1m = v1_pool.tile([128, KC * 128], F16, tag="v1")
                        nc.sync.dma_start(
                            v1m[:], v1t[m].rearrange("p k j -> p (k j)"))

                    gps = [psum.tile([128, w], F32, tag="ps",
                                     name=f"g{hi}_{m}_{b}")
                           for b, (c0, w) in enumerate(blocks)]
                    for k in range(KC):
                        wk = w1m[:, k * 128:(k + 1) * 128]
                        for b, (c0, w) in enumerate(blocks):
                            nc.tensor.matmul(
                                gps[b][:], wk, xts[k][:, c0:c0 + w],
                                start=(k == 0), stop=(k == KC - 1))
                    # silu(gate) -> fp16 tmp frees gate banks while up runs
                    tmps = []
                    for b, (c0, w) in enumerate(blocks):
                        tmp = tmp_pool.tile([128, w], F16, tag="tmp",
                                            name=f"t{hi}_{m}_{b}")
                        nc.scalar.activation(tmp[:], gps[b][:], Silu)
                        tmps.append(tmp)
                    ups = [psum.tile([128, w], F32, tag="ps",
                                     name=f"u{hi}_{m}_{b}")
                           for b, (c0, w) in enumerate(blocks)]
                    for k in range(KC):
                        vk = v1m[:, k * 128:(k + 1) * 128]
                        for b, (c0, w) in enumerate(blocks):
                            nc.tensor.matmul(
                                ups[b][:], vk, xts[k][:, c0:c0 + w],
                                start=(k == 0), stop=(k == KC - 1))
                    pm = p_pool.tile([128, ch], F16, tag="p",
                                     name=f"p{hi}_{m}")
                    for b, (c0, w) in enumerate(blocks):
                        nc.vector.tensor_mul(
                            pm[:, c0:c0 + w], tmps[b][:], ups[b][:])
                    ptiles.append(pm)

                # ---- Phase B: y^T[f, c] = sum_m w2t[m][:, f] P[m]
                for g in range(KC // FGB):
                    yps = [[psum.tile([128, w], F32, tag="ps",
                                      name=f"y{hi}_{g}_{j}_{b}")
                            for b, (c0, w) in enumerate(blocks)]
                           for j in range(FGB)]
                    for m in range(MH):
                        w2m = w2_pool.tile([128, FGB * 128], F16, tag="w2")
                        nc.sync.dma_start(
                            w2m[:],
                            w2t[m][:, g * FGB * 128:(g + 1) * FGB * 128])
                        for j in range(FGB):
                            wj = w2m[:, j * 128:(j + 1) * 128]
                            for b, (c0, w) in enumerate(blocks):
                                nc.tensor.matmul(
                                    yps[j][b][:], wj,
                                    ptiles[m][:, c0:c0 + w],
                                    start=(m == 0), stop=(m == MH - 1))
                    for j in range(FGB):
                        fc = g * FGB + j
                        for b, (c0, w) in enumerate(blocks):
                            ysb = y_pool.tile([128, w], F32, tag="y",
                                              name=f"ys{hi}_{g}_{j}_{b}")
                            nc.vector.tensor_copy(ysb[:], yps[j][b][:])
                            nc.sync.dma_start(
                                yt[fc][:, h0 + c0:h0 + c0 + w], ysb[:])

    nc.compile()
    return nc


def _get_nc(C):
    if C not in _cache:
        _cache[C] = _build(C)
    return _cache[C]


def _expert_inputs(xe, w1e, v1e, w2e, C):
    xt_host = _round_tf32(np.ascontiguousarray(xe.T)).reshape(KC, 128, C)
    w1t_host = _round_tf32(np.ascontiguousarray(
        w1e.reshape(KC, 128, MH, 128).transpose(2, 1, 0, 3)))
    v1t_host = _round_tf32(np.ascontiguousarray(
        v1e.reshape(KC, 128, MH, 128).transpose(2, 1, 0, 3)))
    w2t_host = _round_tf32(np.ascontiguousarray(w2e.T).reshape(MH, 128, F))
    return {"xt": xt_host, "w1t": w1t_host, "v1t": v1t_host, "w2t": w2t_host}


def kernel(hidden_states, top_k_weights, w1, v1, w2, top_k_index):
    from concourse.bass_utils import run_bass_kernel_spmd

    hidden_states = np.asarray(hidden_states)
    top_k_weights = np.asarray(top_k_weights, dtype=np.float32)
    top_k_index = np.asarray(top_k_index)
    w1 = np.asarray(w1, dtype=np.float32)
    v1 = np.asarray(v1, dtype=np.float32)
    w2 = np.asarray(w2, dtype=np.float32)

    B, S, Fdim = hidden_states.shape
    assert Fdim == F
    T = B * S
    x = hidden_states.reshape(T, F).astype(np.float32)

    sels, cws = [], []
    for e in range(E):
        hit = (top_k_index == e)
        sel = np.nonzero(hit.any(axis=1))[0]
        cw = (top_k_weights * hit).sum(axis=1)[sel].astype(np.float32)
        sels.append(sel)
        cws.append(cw)

    max_n = max(len(s) for s in sels)
    C = max(128, ((max_n + 7) // 8) * 8)
    nc = _get_nc(C)

    in_maps = []
    for e in range(E):
        sel = sels[e]
        n = len(sel)
        xe = np.zeros((C, F), np.float32)
        xe[:n] = x[sel]
        in_maps.append(_expert_inputs(xe, w1[e], v1[e], w2[e], C))

    res = run_bass_kernel_spmd(nc, in_maps, core_ids=list(range(E)))

    out = np.zeros((T, F), np.float32)
    for e in range(E):
        sel = sels[e]
        n = len(sel)
        yte = res.results[e]["yt"].reshape(F, C)
        ye = yte[:, :n].T
        out[sel] += cws[e][:, None] * ye
    return out.reshape(B, S, F)

